# revision 30
# baseline (speedup 1.0000x reference)
"""Trainium2 Bass kernel for causal multi-head attention.

Problem: B=4, S=2048, D=512, H=8 heads (head_dim 64), causal mask.
  q = x @ Wq.T + bq ; k = x @ Wk.T + bk ; v = x @ Wv.T + bv
  att = softmax(mask(q k^T / sqrt(64))) @ v ; out = att @ Wo.T + bo

Sharding: 8 cores = (batch b in 0..3) x (head-group hg in 0..1, 4 heads each).
Each core computes its 4 heads' Q/K/V projections, attention, and a partial
out-projection (contribution of its head block). Host sums the two partials
per batch and adds bo. No collectives needed.

Device-side layout tricks (all matmuls contract along SBUF partitions):
 - host feeds x transposed (xT [512, S]) with an extra ones-row, and weights
   pre-transposed with the bias folded in as an extra contraction row.
 - scores are computed TRANSPOSED (ST[k, q]) so that exp(ST) is directly the
   stationary->moving operand needed by the attention*V matmul, and the
   softmax denominator falls out of the same matmul via a ones-column
   interleaved into V. Softmax therefore needs no reductions at all.
   (No max-subtraction: weights are scaled so logits are O(1); exp is safe.)
 - causal structure is exploited exactly: k-tiles above the diagonal are
   skipped, band k-tiles only produce their valid q columns, and only the
   leading 128-column triangle of each band range is masked (one static
   [128,128] 0/1 tile).
 - matmul operands are bf16 (fp32 matmul is 4x slower on trn2); accumulation
   is fp32 in PSUM, softmax normalization fp32.
 - 1/sumexp: DVE reciprocal costs 6 cycles per free-dim element, so the
   [1,512] sumexp row is 32x32-block-transposed first, reciprocal'd on the
   16 real elements per partition, and transposed back (~1.4us vs 3.3us).

The mask input is verified on the host: if it is exactly the causal mask the
fast path runs; otherwise a generic variant runs that reads a host-prepared
transposed multiplicative mask from DRAM.
"""

import sys

import numpy as np

for _p in ("/opt/trn_rl_repo",):
    if _p not in sys.path:
        sys.path.insert(0, _p)

import ml_dtypes  # noqa: E402

import concourse.bass as bass  # noqa: E402
import concourse.tile as tile  # noqa: E402
from concourse import bacc, mybir  # noqa: E402

B, S, D, H = 4, 2048, 512, 8
HD = D // H  # 64
P = 128
HG = 4  # heads per core
DG = HG * HD  # 256 per-core head dims
QB = 512  # q-block (matmul moving free dim)
NQB = S // QB  # 4
NKT = S // P  # 16 k-tiles
KTQ = QB // P  # 4 k-tiles per q-block (diagonal band width)
NET = D // P  # 4 e-tiles (contraction tiles for projections)
VW = HG * (HD + 1)  # 260: V with an interleaved ones-column per head

F32 = mybir.dt.float32
BF16 = mybir.dt.bfloat16
NPBF16 = ml_dtypes.bfloat16

_BUILT = {}


def _build_nc(causal: bool):
    """Build (and bacc-compile) the SPMD single-core program."""
    nc = bacc.Bacc("TRN2", target_bir_lowering=False, debug=False, num_devices=8)

    xT_d = nc.dram_tensor("xT", [D + 1, S], BF16, kind="ExternalInput").ap()
    wq_d = nc.dram_tensor("wq", [D, DG], BF16, kind="ExternalInput").ap()
    bq_d = nc.dram_tensor("bqv", [DG, 1], F32, kind="ExternalInput").ap()
    wk_d = nc.dram_tensor("wk", [D, DG], BF16, kind="ExternalInput").ap()
    wv_d = nc.dram_tensor("wv", [D + 1, VW], BF16, kind="ExternalInput").ap()
    wo_d = nc.dram_tensor("wo", [DG, D], BF16, kind="ExternalInput").ap()
    if causal:
        bm_d = nc.dram_tensor("bm", [P, KTQ * QB], BF16, kind="ExternalInput").ap()
    else:
        mt_d = nc.dram_tensor("mt", [HG, S, S], BF16, kind="ExternalInput").ap()
    out_d = nc.dram_tensor("out", [D, S], F32, kind="ExternalOutput").ap()

    EXP = mybir.ActivationFunctionType.Exp
    LN = mybir.ActivationFunctionType.Ln

    with tile.TileContext(nc) as tc:
        with (
            tc.tile_pool(name="consts", bufs=1) as consts,
            tc.tile_pool(name="work", bufs=3) as work,
            tc.tile_pool(name="attn", bufs=3) as attnp,
            tc.tile_pool(name="small", bufs=3) as small,
            tc.tile_pool(name="pmm", bufs=2, space="PSUM") as pmm,
            tc.tile_pool(name="pst", bufs=1, space="PSUM") as pst,
            tc.tile_pool(name="patt", bufs=2, space="PSUM") as patt,
        ):
            # ---- load persistent operands ----
            # weights first (small), then x sliced per s-block so the first
            # projection matmuls can start as soon as the first slices land.
            xts = [
                consts.tile([P, S], BF16, tag=f"xt{et}", name=f"xts{et}")
                for et in range(NET)
            ]

            # DMA order: wk + first x-block first (the first projection
            # matmuls need exactly these), then remaining weights, then the
            # rest of x; wo/bm (needed latest) last.
            wk_t = []
            for et in range(NET):
                tk = consts.tile([P, DG], BF16, tag=f"wk{et}")
                nc.sync.dma_start(out=tk, in_=wk_d[et * P : (et + 1) * P, :])
                wk_t.append(tk)
            for et in range(NET):
                nc.sync.dma_start(
                    out=xts[et][:, 0:QB], in_=xT_d[et * P : (et + 1) * P, 0:QB]
                )
            wq_t, wv_t = [], []
            for et in range(NET):
                tq = consts.tile([P, DG], BF16, tag=f"wq{et}")
                nc.sync.dma_start(out=tq, in_=wq_d[et * P : (et + 1) * P, :])
                wq_t.append(tq)
            bq_sb = []
            for j in range(2):
                t = consts.tile([P, 1], F32, tag=f"bq{j}")
                nc.sync.dma_start(out=t, in_=bq_d[j * P : (j + 1) * P, :])
                bq_sb.append(t)
            for et in range(NET):
                tv = consts.tile([P, VW], BF16, tag=f"wv{et}")
                nc.sync.dma_start(out=tv, in_=wv_d[et * P : (et + 1) * P, :])
                wv_t.append(tv)
            wvb = consts.tile([1, VW], BF16, tag="wvb")
            nc.sync.dma_start(out=wvb, in_=wv_d[D : D + 1, :])
            xon = consts.tile([1, S], BF16, tag="xon")
            nc.sync.dma_start(out=xon, in_=xT_d[D : D + 1, :])
            for sb in range(1, S // QB):
                ssl = slice(sb * QB, (sb + 1) * QB)
                for et in range(NET):
                    nc.sync.dma_start(
                        out=xts[et][:, ssl], in_=xT_d[et * P : (et + 1) * P, ssl]
                    )
            wo_t = []
            for j in range(2):
                t = consts.tile([P, D], BF16, tag=f"wo{j}")
                nc.sync.dma_start(out=t, in_=wo_d[j * P : (j + 1) * P, :])
                wo_t.append(t)
            if causal:
                bm = consts.tile([P, KTQ * QB], BF16, tag="bm")
                nc.sync.dma_start(out=bm, in_=bm_d)

            # ---- Q/K/V projections, emitted per s-block so attention on the
            # first q-block can start while later blocks still project.
            # QT/KT transposed [dg, s]; V natural [s, (v|1) interleaved].
            QT = [consts.tile([P, S], BF16, tag=f"qt{i}", name=f"QT{i}") for i in range(2)]
            KT = [consts.tile([P, S], BF16, tag=f"kt{i}", name=f"KT{i}") for i in range(2)]
            V = [
                consts.tile([P, VW], BF16, tag=f"v{st}", name=f"Vt{st}")
                for st in range(NKT)
            ]
            for sb in range(S // QB):
                ssl = slice(sb * QB, (sb + 1) * QB)
                for dgt in range(2):
                    dsl = slice(dgt * P, (dgt + 1) * P)
                    ps2 = pmm.tile([P, QB], F32, tag="mm")
                    for et in range(NET):
                        nc.tensor.matmul(
                            ps2,
                            wk_t[et][:, dsl],
                            xts[et][:, ssl],
                            start=(et == 0),
                            stop=(et == NET - 1),
                        )
                    nc.vector.tensor_copy(KT[dgt][:, ssl], ps2)

                    ps = pmm.tile([P, QB], F32, tag="mm")
                    for et in range(NET):
                        nc.tensor.matmul(
                            ps,
                            wq_t[et][:, dsl],
                            xts[et][:, ssl],
                            start=(et == 0),
                            stop=(et == NET - 1),
                        )
                    # fold bq in during the PSUM->SBUF cast
                    nc.vector.tensor_scalar_add(QT[dgt][:, ssl], ps, bq_sb[dgt])

                for st in range(4 * sb, 4 * sb + 4):
                    ksl = slice(st * P, (st + 1) * P)
                    ps = pmm.tile([P, VW], F32, tag="mm")
                    for et in range(NET):
                        nc.tensor.matmul(
                            ps, xts[et][:, ksl], wv_t[et], start=(et == 0), stop=False
                        )
                    nc.tensor.matmul(ps, xon[:, ksl], wvb, start=False, stop=True)
                    nc.vector.tensor_copy(V[st], ps)

            # ---- attention + out-projection, per q-block ----
            # Head PAIRS are interleaved: head 2*hp uses PE rows 0..63, head
            # 2*hp+1 rows 64..127, so consecutive score matmuls hit different
            # row-groups (concurrent execution + LDWEIGHTS pulled ahead).
            for qb in range(NQB):
                qsl = slice(qb * QB, (qb + 1) * QB)
                attn_t = [attnp.tile([P, QB], BF16, tag=f"attn{i}", name=f"attn{i}_{qb}") for i in range(2)]
                for hp in range(2):
                    hA, hB = 2 * hp, 2 * hp + 1
                    dgt = hp
                    rA, rB = slice(0, HD), slice(HD, 2 * HD)
                    attps = [
                        patt.tile([P, QB], F32, tag="att", name=f"att{qb}_{h}")
                        for h in (hA, hB)
                    ]
                    # Each group packs score columns for up to 2 k-tiles in
                    # one wide PSUM tile: (kt, col in wide tile, q-offset in
                    # the q-block, width, needs-triangle-mask). Causal band
                    # k-tiles only produce their VALID q columns; the leading
                    # 128 columns of each band range form the causal triangle.
                    if causal:
                        groups = [
                            [(2 * ip, 0, 0, QB, False), (2 * ip + 1, QB, 0, QB, False)]
                            for ip in range(qb * KTQ // 2)
                        ]
                        b0 = qb * KTQ
                        groups.append(
                            [(b0, 0, 0, QB, True), (b0 + 1, QB, P, QB - P, True)]
                        )
                        groups.append(
                            [
                                (b0 + 2, 0, 2 * P, QB - 2 * P, True),
                                (b0 + 3, QB - 2 * P, 3 * P, QB - 3 * P, True),
                            ]
                        )
                    else:
                        groups = [
                            [(2 * ip, 0, 0, QB, False), (2 * ip + 1, QB, 0, QB, False)]
                            for ip in range(NKT // 2)
                        ]
                    last_kt = groups[-1][-1][0]
                    for grp in groups:
                        stA = pst.tile([P, 2 * QB], F32, tag="stA")
                        stB = pst.tile([P, 2 * QB], F32, tag="stB")
                        for kt, col, qo, w, _tri in grp:
                            jsl = slice(col, col + w)
                            ksl = slice(kt * P, (kt + 1) * P)
                            qsub = slice(qb * QB + qo, (qb + 1) * QB)
                            nc.tensor.matmul(
                                stA[:, jsl], KT[dgt][rA, ksl], QT[dgt][rA, qsub],
                                start=True, stop=True, tile_position=(0, 0),
                            )
                            nc.tensor.matmul(
                                stB[:, jsl], KT[dgt][rB, ksl], QT[dgt][rB, qsub],
                                start=True, stop=True, tile_position=(64, 0),
                            )
                        wtot = grp[-1][1] + grp[-1][3]
                        exA = work.tile([P, 2 * QB], BF16, tag="exA")
                        exB = work.tile([P, 2 * QB], BF16, tag="exB")
                        # scores are q.k / sqrt(64): fold 1/8 into the exp
                        nc.scalar.activation(exA[:, :wtot], stA[:, :wtot], EXP, scale=0.125)
                        nc.scalar.activation(exB[:, :wtot], stB[:, :wtot], EXP, scale=0.125)
                        for kt, col, qo, w, tri in grp:
                            if tri:
                                tsl = slice(col, col + P)
                                nc.vector.tensor_mul(exA[:, tsl], exA[:, tsl], bm[:, 0:P])
                                nc.vector.tensor_mul(exB[:, tsl], exB[:, tsl], bm[:, 0:P])
                            elif not causal:
                                for h, ex in ((hA, exA), (hB, exB)):
                                    mtile = work.tile([P, QB], BF16, tag="mt")
                                    nc.sync.dma_start(
                                        out=mtile,
                                        in_=mt_d[h, kt * P : (kt + 1) * P, qsl],
                                    )
                                    nc.vector.tensor_mul(
                                        ex[:, col : col + w], ex[:, col : col + w], mtile
                                    )
                        for kt, col, qo, w, _tri in grp:
                            first = kt == 0
                            last = kt == last_kt
                            for h, ex, aps in ((hA, exA, attps[0]), (hB, exB, attps[1])):
                                nc.tensor.matmul(
                                    aps[0 : HD + 1, qo : qo + w],
                                    V[kt][:, h * (HD + 1) : (h + 1) * (HD + 1)],
                                    ex[:, col : col + w],
                                    start=first, stop=last,
                                )
                    # normalize: rows 0..63 are sum(exp * v), row 64 is sum(exp)
                    # DVE reciprocal costs 6 cycles per FREE-dim element, so
                    # 1/sumexp on the [1,512] row is 3.3us. Instead transpose
                    # 32x32 blocks (row -> strided columns), reciprocal just
                    # the 16 real elements per partition (~0.1us), transpose
                    # back. Rows 65..95 of the PSUM tile are never written;
                    # their junk is copied around but only row 0 of t3 is read.
                    for h, aps, rsl in ((hA, attps[0], rA), (hB, attps[1], rB)):
                        # copy PSUM->SBUF immediately so the PSUM bank frees
                        # for the next head pair; normalize from SBUF.
                        au = small.tile([HD + 32, QB], F32, tag="au")
                        nc.vector.tensor_copy(au[0 : HD + 1, :], aps[0 : HD + 1, :])
                        t1 = small.tile([32, QB], F32, tag="t1")
                        nc.vector.transpose(t1, au[HD : HD + 32, :])
                        t2 = small.tile([32, QB], F32, tag="t2")
                        nc.vector.reciprocal(
                            out=t2.rearrange("p (j c) -> p j c", c=32)[:, :, 0],
                            in_=t1.rearrange("p (j c) -> p j c", c=32)[:, :, 0],
                        )
                        t3 = small.tile([32, QB], F32, tag="t3")
                        nc.vector.transpose(t3, t2)
                        rb = small.tile([HD, QB], F32, tag="rb")
                        nc.gpsimd.partition_broadcast(rb, t3[0:1, :])
                        nc.vector.tensor_mul(attn_t[dgt][rsl, :], au[0:HD, :], rb)

                # partial out-projection for this q-block: out[e, q]
                for et in range(NET):
                    esl = slice(et * P, (et + 1) * P)
                    ops = pmm.tile([P, QB], F32, tag="mm")
                    nc.tensor.matmul(
                        ops, wo_t[0][:, esl], attn_t[0], start=True, stop=False
                    )
                    nc.tensor.matmul(
                        ops, wo_t[1][:, esl], attn_t[1], start=False, stop=True
                    )
                    ost = work.tile([P, QB], F32, tag="ost")
                    nc.vector.tensor_copy(ost, ops)
                    nc.sync.dma_start(out=out_d[esl, qsl], in_=ost)

    nc.compile()
    return nc


def _get_nc(causal: bool):
    if causal not in _BUILT:
        _BUILT[causal] = _build_nc(causal)
    return _BUILT[causal]


def _band_mask():
    """[128, KTQ*QB] 0/1 tiles: tile oi valid iff qi >= ki + oi*128."""
    ki = np.arange(P)[:, None]
    qi = np.arange(QB)[None, :]
    tiles = [(qi >= ki + oi * P).astype(np.float32) for oi in range(KTQ)]
    return np.concatenate(tiles, axis=1).astype(NPBF16)


def _prep_core_inputs(x, mask, Wq, bq, Wk, Wv, bv, Wo, causal):
    """Build the 8 per-core input maps (bf16, pre-transposed, biases folded)."""
    ones_row = np.ones((1, S), np.float32)
    bm = _band_mask()
    in_maps = []
    for c in range(8):
        b, hg = c // 2, c % 2
        h0, e0 = hg * HG, hg * DG
        xt = np.concatenate([x[b].T, ones_row], axis=0).astype(NPBF16)
        wq = Wq[e0 : e0 + DG, :].T.astype(NPBF16)
        bqv = np.ascontiguousarray(bq[e0 : e0 + DG][:, None], dtype=np.float32)
        wk = Wk[e0 : e0 + DG, :].T.astype(NPBF16)
        # V weights with bias row; ones-column interleaved per head for the
        # softmax denominator (weight 0, bias 1).
        wv = np.zeros((D + 1, VW), np.float32)
        for h in range(HG):
            eh = e0 + h * HD
            wv[:D, h * (HD + 1) : h * (HD + 1) + HD] = Wv[eh : eh + HD, :].T
            wv[D, h * (HD + 1) : h * (HD + 1) + HD] = bv[eh : eh + HD]
            wv[D, h * (HD + 1) + HD] = 1.0
        wo = Wo[:, e0 : e0 + DG].T.astype(NPBF16)
        m = {
            "xT": xt,
            "wq": wq,
            "bqv": bqv,
            "wk": wk,
            "wv": wv.astype(NPBF16),
            "wo": wo,
        }
        if causal:
            m["bm"] = bm
        else:
            # transposed multiplicative mask per local head: mt[h, k, q]
            mt = np.ascontiguousarray(
                mask[b, h0 : h0 + HG].transpose(0, 2, 1)
            ).astype(NPBF16)
            m["mt"] = mt
        in_maps.append(m)
    return in_maps


def kernel(**inputs):
    from concourse.bass_utils import run_bass_kernel_spmd

    x = np.asarray(inputs["x"], dtype=np.float32)
    mask = np.asarray(inputs["mask"])
    Wq = np.asarray(inputs["Wq"], dtype=np.float32)
    bq = np.asarray(inputs["bq"], dtype=np.float32)
    Wk = np.asarray(inputs["Wk"], dtype=np.float32)
    Wv = np.asarray(inputs["Wv"], dtype=np.float32)
    bv = np.asarray(inputs["bv"], dtype=np.float32)
    Wo = np.asarray(inputs["Wo"], dtype=np.float32)
    bo = np.asarray(inputs["bo"], dtype=np.float32)
    # bk is softmax-invariant (adds a per-query constant to all logits in a
    # row), so it is deliberately not used.

    causal = bool(
        (mask == np.tril(np.ones((S, S), dtype=bool))[None, None]).all()
    )

    nc = _get_nc(causal)
    in_maps = _prep_core_inputs(x, mask, Wq, bq, Wk, Wv, bv, Wo, causal)
    res = run_bass_kernel_spmd(nc, in_maps, core_ids=list(range(8)))
    out = np.empty((B, S, D), np.float32)
    for b in range(B):
        partial = res.results[2 * b]["out"] + res.results[2 * b + 1]["out"]
        out[b] = partial.T + bo[None, :]
    return out


# revision 31
# speedup vs baseline: 1.1424x; 1.1424x over previous
"""Trainium2 Bass kernel for causal multi-head attention.

Problem: B=4, S=2048, D=512, H=8 heads (head_dim 64), causal mask.
  q = x @ Wq.T + bq ; k = x @ Wk.T + bk ; v = x @ Wv.T + bv
  att = softmax(mask(q k^T / sqrt(64))) @ v ; out = att @ Wo.T + bo

Sharding: 8 cores = (batch b in 0..3) x (head-group hg in 0..1, 4 heads each).
Each core computes its 4 heads' Q/K/V projections, attention, and a partial
out-projection (contribution of its head block). Host sums the two partials
per batch and adds bo. No collectives needed.

Device-side layout tricks (all matmuls contract along SBUF partitions):
 - host feeds x transposed (xT [512, S]) with an extra ones-row, and weights
   pre-transposed with the bias folded in as an extra contraction row.
 - scores are computed TRANSPOSED (ST[k, q]) so that exp(ST) is directly the
   stationary->moving operand needed by the attention*V matmul, and the
   softmax denominator falls out of the same matmul via a ones-column
   interleaved into V. Softmax therefore needs no reductions at all.
   (No max-subtraction: weights are scaled so logits are O(1); exp is safe.)
 - causal structure is exploited exactly: k-tiles above the diagonal are
   skipped, band k-tiles only produce their valid q columns, and only the
   leading 128-column triangle of each band range is masked (one static
   [128,128] 0/1 tile).
 - matmul operands are bf16 (fp32 matmul is 4x slower on trn2); accumulation
   is fp32 in PSUM, softmax normalization fp32.
 - 1/sumexp: DVE reciprocal costs 6 cycles per free-dim element, so the
   [1,512] sumexp row is 32x32-block-transposed first, reciprocal'd on the
   16 real elements per partition, and transposed back (~1.4us vs 3.3us).

The mask input is verified on the host: if it is exactly the causal mask the
fast path runs; otherwise a generic variant runs that reads a host-prepared
transposed multiplicative mask from DRAM.
"""

import sys

import numpy as np

for _p in ("/opt/trn_rl_repo",):
    if _p not in sys.path:
        sys.path.insert(0, _p)

import ml_dtypes  # noqa: E402

import concourse.bass as bass  # noqa: E402
import concourse.tile as tile  # noqa: E402
from concourse import bacc, mybir  # noqa: E402

B, S, D, H = 4, 2048, 512, 8
HD = D // H  # 64
P = 128
HG = 4  # heads per core
DG = HG * HD  # 256 per-core head dims
QB = 512  # q-block (matmul moving free dim)
NQB = S // QB  # 4
NKT = S // P  # 16 k-tiles
KTQ = QB // P  # 4 k-tiles per q-block (diagonal band width)
NET = D // P  # 4 e-tiles (contraction tiles for projections)
VW = HG * (HD + 1)  # 260: V with an interleaved ones-column per head

F32 = mybir.dt.float32
BF16 = mybir.dt.bfloat16
NPBF16 = ml_dtypes.bfloat16

_BUILT = {}


def _build_nc(causal: bool):
    """Build (and bacc-compile) the SPMD single-core program."""
    nc = bacc.Bacc("TRN2", target_bir_lowering=False, debug=False, num_devices=8)

    xT_d = nc.dram_tensor("xT", [D + 1, S], BF16, kind="ExternalInput").ap()
    wq_d = nc.dram_tensor("wq", [D, DG], BF16, kind="ExternalInput").ap()
    bq_d = nc.dram_tensor("bqv", [DG, 1], F32, kind="ExternalInput").ap()
    wk_d = nc.dram_tensor("wk", [D, DG], BF16, kind="ExternalInput").ap()
    wv_d = nc.dram_tensor("wv", [D + 1, VW], BF16, kind="ExternalInput").ap()
    wo_d = nc.dram_tensor("wo", [DG, D], BF16, kind="ExternalInput").ap()
    if causal:
        bm_d = nc.dram_tensor("bm", [P, KTQ * QB], BF16, kind="ExternalInput").ap()
    else:
        mt_d = nc.dram_tensor("mt", [HG, S, S], BF16, kind="ExternalInput").ap()
    out_d = nc.dram_tensor("out", [D, S], F32, kind="ExternalOutput").ap()

    EXP = mybir.ActivationFunctionType.Exp
    LN = mybir.ActivationFunctionType.Ln

    with tile.TileContext(nc) as tc:
        with (
            tc.tile_pool(name="consts", bufs=1) as consts,
            tc.tile_pool(name="work", bufs=3) as work,
            tc.tile_pool(name="attn", bufs=3) as attnp,
            tc.tile_pool(name="small", bufs=3) as small,
            tc.tile_pool(name="pmm", bufs=2, space="PSUM") as pmm,
            tc.tile_pool(name="pst", bufs=1, space="PSUM") as pst,
            tc.tile_pool(name="patt", bufs=2, space="PSUM") as patt,
        ):
            # ---- load persistent operands ----
            # weights first (small), then x sliced per s-block so the first
            # projection matmuls can start as soon as the first slices land.
            xts = [
                consts.tile([P, S], BF16, tag=f"xt{et}", name=f"xts{et}")
                for et in range(NET)
            ]

            # DMA order: wk + first x-block first (the first projection
            # matmuls need exactly these), then remaining weights, then the
            # rest of x; wo/bm (needed latest) last.
            wk_t = []
            for et in range(NET):
                tk = consts.tile([P, DG], BF16, tag=f"wk{et}")
                nc.sync.dma_start(out=tk, in_=wk_d[et * P : (et + 1) * P, :])
                wk_t.append(tk)
            for et in range(NET):
                nc.sync.dma_start(
                    out=xts[et][:, 0:QB], in_=xT_d[et * P : (et + 1) * P, 0:QB]
                )
            wq_t, wv_t = [], []
            for et in range(NET):
                tq = consts.tile([P, DG], BF16, tag=f"wq{et}")
                nc.sync.dma_start(out=tq, in_=wq_d[et * P : (et + 1) * P, :])
                wq_t.append(tq)
            bq_sb = []
            for j in range(2):
                t = consts.tile([P, 1], F32, tag=f"bq{j}")
                nc.sync.dma_start(out=t, in_=bq_d[j * P : (j + 1) * P, :])
                bq_sb.append(t)
            for et in range(NET):
                tv = consts.tile([P, VW], BF16, tag=f"wv{et}")
                nc.sync.dma_start(out=tv, in_=wv_d[et * P : (et + 1) * P, :])
                wv_t.append(tv)
            wvb = consts.tile([1, VW], BF16, tag="wvb")
            nc.sync.dma_start(out=wvb, in_=wv_d[D : D + 1, :])
            xon = consts.tile([1, S], BF16, tag="xon")
            nc.sync.dma_start(out=xon, in_=xT_d[D : D + 1, :])
            for sb in range(1, S // QB):
                ssl = slice(sb * QB, (sb + 1) * QB)
                for et in range(NET):
                    nc.sync.dma_start(
                        out=xts[et][:, ssl], in_=xT_d[et * P : (et + 1) * P, ssl]
                    )
            wo_t = []
            for j in range(2):
                t = consts.tile([P, D], BF16, tag=f"wo{j}")
                nc.sync.dma_start(out=t, in_=wo_d[j * P : (j + 1) * P, :])
                wo_t.append(t)
            if causal:
                bm = consts.tile([P, KTQ * QB], BF16, tag="bm")
                nc.sync.dma_start(out=bm, in_=bm_d)

            # ---- Q/K/V projections, emitted per s-block so attention on the
            # first q-block can start while later blocks still project.
            # QT/KT transposed [dg, s]; V natural [s, (v|1) interleaved].
            QT = [consts.tile([P, S], BF16, tag=f"qt{i}", name=f"QT{i}") for i in range(2)]
            KT = [consts.tile([P, S], BF16, tag=f"kt{i}", name=f"KT{i}") for i in range(2)]
            V = [
                consts.tile([P, VW], BF16, tag=f"v{st}", name=f"Vt{st}")
                for st in range(NKT)
            ]
            for sb in range(S // QB):
                ssl = slice(sb * QB, (sb + 1) * QB)
                for dgt in range(2):
                    dsl = slice(dgt * P, (dgt + 1) * P)
                    ps2 = pmm.tile([P, QB], F32, tag="mm")
                    for et in range(NET):
                        nc.tensor.matmul(
                            ps2,
                            wk_t[et][:, dsl],
                            xts[et][:, ssl],
                            start=(et == 0),
                            stop=(et == NET - 1),
                        )
                    nc.vector.tensor_copy(KT[dgt][:, ssl], ps2)

                    ps = pmm.tile([P, QB], F32, tag="mm")
                    for et in range(NET):
                        nc.tensor.matmul(
                            ps,
                            wq_t[et][:, dsl],
                            xts[et][:, ssl],
                            start=(et == 0),
                            stop=(et == NET - 1),
                        )
                    # fold bq in during the PSUM->SBUF cast
                    nc.vector.tensor_scalar_add(QT[dgt][:, ssl], ps, bq_sb[dgt])

                for st in range(4 * sb, 4 * sb + 4):
                    ksl = slice(st * P, (st + 1) * P)
                    ps = pmm.tile([P, VW], F32, tag="mm")
                    for et in range(NET):
                        nc.tensor.matmul(
                            ps, xts[et][:, ksl], wv_t[et], start=(et == 0), stop=False
                        )
                    nc.tensor.matmul(ps, xon[:, ksl], wvb, start=False, stop=True)
                    nc.vector.tensor_copy(V[st], ps)

            # ---- attention + out-projection, per q-block ----
            # Head PAIRS are interleaved: head 2*hp uses PE rows 0..63, head
            # 2*hp+1 rows 64..127, so consecutive score matmuls hit different
            # row-groups (concurrent execution + LDWEIGHTS pulled ahead).
            for qb in range(NQB):
                qsl = slice(qb * QB, (qb + 1) * QB)
                attn_t = [attnp.tile([P, QB], BF16, tag=f"attn{i}", name=f"attn{i}_{qb}") for i in range(2)]
                for hp in range(2):
                    hA, hB = 2 * hp, 2 * hp + 1
                    dgt = hp
                    rA, rB = slice(0, HD), slice(HD, 2 * HD)
                    attps = [
                        patt.tile([P, QB], F32, tag="att", name=f"att{qb}_{h}")
                        for h in (hA, hB)
                    ]
                    # Each group packs score columns for up to 2 k-tiles in
                    # one wide PSUM tile: (kt, col in wide tile, q-offset in
                    # the q-block, width, needs-triangle-mask). Causal band
                    # k-tiles only produce their VALID q columns; the leading
                    # 128 columns of each band range form the causal triangle.
                    if causal:
                        groups = [
                            [(2 * ip, 0, 0, QB, False), (2 * ip + 1, QB, 0, QB, False)]
                            for ip in range(qb * KTQ // 2)
                        ]
                        b0 = qb * KTQ
                        groups.append(
                            [(b0, 0, 0, QB, True), (b0 + 1, QB, P, QB - P, True)]
                        )
                        groups.append(
                            [
                                (b0 + 2, 0, 2 * P, QB - 2 * P, True),
                                (b0 + 3, QB - 2 * P, 3 * P, QB - 3 * P, True),
                            ]
                        )
                    else:
                        groups = [
                            [(2 * ip, 0, 0, QB, False), (2 * ip + 1, QB, 0, QB, False)]
                            for ip in range(NKT // 2)
                        ]
                    last_kt = groups[-1][-1][0]
                    # software pipeline: AV matmuls for group g are emitted
                    # after the scores+exp of group g+1, so the PE never
                    # head-of-line blocks on exp(g) — it streams AV(g-1)
                    # while the Scalar engine works on exp(g).
                    pending = None
                    for grp in groups:
                        stA = pst.tile([P, 2 * QB], F32, tag="stA")
                        stB = pst.tile([P, 2 * QB], F32, tag="stB")
                        for kt, col, qo, w, _tri in grp:
                            jsl = slice(col, col + w)
                            ksl = slice(kt * P, (kt + 1) * P)
                            qsub = slice(qb * QB + qo, (qb + 1) * QB)
                            nc.tensor.matmul(
                                stA[:, jsl], KT[dgt][rA, ksl], QT[dgt][rA, qsub],
                                start=True, stop=True, tile_position=(0, 0),
                            )
                            nc.tensor.matmul(
                                stB[:, jsl], KT[dgt][rB, ksl], QT[dgt][rB, qsub],
                                start=True, stop=True, tile_position=(64, 0),
                            )
                        wtot = grp[-1][1] + grp[-1][3]
                        exA = work.tile([P, 2 * QB], BF16, tag="exA")
                        exB = work.tile([P, 2 * QB], BF16, tag="exB")
                        # scores are q.k / sqrt(64): fold 1/8 into the exp
                        nc.scalar.activation(exA[:, :wtot], stA[:, :wtot], EXP, scale=0.125)
                        nc.scalar.activation(exB[:, :wtot], stB[:, :wtot], EXP, scale=0.125)
                        for kt, col, qo, w, tri in grp:
                            if tri:
                                tsl = slice(col, col + P)
                                nc.vector.tensor_mul(exA[:, tsl], exA[:, tsl], bm[:, 0:P])
                                nc.vector.tensor_mul(exB[:, tsl], exB[:, tsl], bm[:, 0:P])
                            elif not causal:
                                for h, ex in ((hA, exA), (hB, exB)):
                                    mtile = work.tile([P, QB], BF16, tag="mt")
                                    nc.sync.dma_start(
                                        out=mtile,
                                        in_=mt_d[h, kt * P : (kt + 1) * P, qsl],
                                    )
                                    nc.vector.tensor_mul(
                                        ex[:, col : col + w], ex[:, col : col + w], mtile
                                    )
                        if pending is not None:
                            pgrp, pexA, pexB = pending
                            for kt, col, qo, w, _tri in pgrp:
                                for h, ex, aps in ((hA, pexA, attps[0]), (hB, pexB, attps[1])):
                                    nc.tensor.matmul(
                                        aps[0 : HD + 1, qo : qo + w],
                                        V[kt][:, h * (HD + 1) : (h + 1) * (HD + 1)],
                                        ex[:, col : col + w],
                                        start=(kt == 0), stop=(kt == last_kt),
                                    )
                        pending = (grp, exA, exB)
                    pgrp, pexA, pexB = pending
                    for kt, col, qo, w, _tri in pgrp:
                        for h, ex, aps in ((hA, pexA, attps[0]), (hB, pexB, attps[1])):
                            nc.tensor.matmul(
                                aps[0 : HD + 1, qo : qo + w],
                                V[kt][:, h * (HD + 1) : (h + 1) * (HD + 1)],
                                ex[:, col : col + w],
                                start=(kt == 0), stop=(kt == last_kt),
                            )
                    # normalize: rows 0..63 are sum(exp * v), row 64 is sum(exp)
                    # DVE reciprocal costs 6 cycles per FREE-dim element, so
                    # 1/sumexp on the [1,512] row is 3.3us. Instead transpose
                    # 32x32 blocks (row -> strided columns), reciprocal just
                    # the 16 real elements per partition (~0.1us), transpose
                    # back. Rows 65..95 of the PSUM tile are never written;
                    # their junk is copied around but only row 0 of t3 is read.
                    for h, aps, rsl in ((hA, attps[0], rA), (hB, attps[1], rB)):
                        # copy PSUM->SBUF immediately so the PSUM bank frees
                        # for the next head pair; normalize from SBUF.
                        au = small.tile([HD + 32, QB], F32, tag="au")
                        nc.vector.tensor_copy(au[0 : HD + 1, :], aps[0 : HD + 1, :])
                        t1 = small.tile([32, QB], F32, tag="t1")
                        nc.vector.transpose(t1, au[HD : HD + 32, :])
                        t2 = small.tile([32, QB], F32, tag="t2")
                        nc.vector.reciprocal(
                            out=t2.rearrange("p (j c) -> p j c", c=32)[:, :, 0],
                            in_=t1.rearrange("p (j c) -> p j c", c=32)[:, :, 0],
                        )
                        t3 = small.tile([32, QB], F32, tag="t3")
                        nc.vector.transpose(t3, t2)
                        rb = small.tile([HD, QB], F32, tag="rb")
                        nc.gpsimd.partition_broadcast(rb, t3[0:1, :])
                        nc.vector.tensor_mul(attn_t[dgt][rsl, :], au[0:HD, :], rb)

                # partial out-projection for this q-block: out[e, q]
                for et in range(NET):
                    esl = slice(et * P, (et + 1) * P)
                    ops = pmm.tile([P, QB], F32, tag="mm")
                    nc.tensor.matmul(
                        ops, wo_t[0][:, esl], attn_t[0], start=True, stop=False
                    )
                    nc.tensor.matmul(
                        ops, wo_t[1][:, esl], attn_t[1], start=False, stop=True
                    )
                    ost = work.tile([P, QB], F32, tag="ost")
                    nc.vector.tensor_copy(ost, ops)
                    nc.sync.dma_start(out=out_d[esl, qsl], in_=ost)

    nc.compile()
    return nc


def _get_nc(causal: bool):
    if causal not in _BUILT:
        _BUILT[causal] = _build_nc(causal)
    return _BUILT[causal]


def _band_mask():
    """[128, KTQ*QB] 0/1 tiles: tile oi valid iff qi >= ki + oi*128."""
    ki = np.arange(P)[:, None]
    qi = np.arange(QB)[None, :]
    tiles = [(qi >= ki + oi * P).astype(np.float32) for oi in range(KTQ)]
    return np.concatenate(tiles, axis=1).astype(NPBF16)


def _prep_core_inputs(x, mask, Wq, bq, Wk, Wv, bv, Wo, causal):
    """Build the 8 per-core input maps (bf16, pre-transposed, biases folded)."""
    ones_row = np.ones((1, S), np.float32)
    bm = _band_mask()
    in_maps = []
    for c in range(8):
        b, hg = c // 2, c % 2
        h0, e0 = hg * HG, hg * DG
        xt = np.concatenate([x[b].T, ones_row], axis=0).astype(NPBF16)
        wq = Wq[e0 : e0 + DG, :].T.astype(NPBF16)
        bqv = np.ascontiguousarray(bq[e0 : e0 + DG][:, None], dtype=np.float32)
        wk = Wk[e0 : e0 + DG, :].T.astype(NPBF16)
        # V weights with bias row; ones-column interleaved per head for the
        # softmax denominator (weight 0, bias 1).
        wv = np.zeros((D + 1, VW), np.float32)
        for h in range(HG):
            eh = e0 + h * HD
            wv[:D, h * (HD + 1) : h * (HD + 1) + HD] = Wv[eh : eh + HD, :].T
            wv[D, h * (HD + 1) : h * (HD + 1) + HD] = bv[eh : eh + HD]
            wv[D, h * (HD + 1) + HD] = 1.0
        wo = Wo[:, e0 : e0 + DG].T.astype(NPBF16)
        m = {
            "xT": xt,
            "wq": wq,
            "bqv": bqv,
            "wk": wk,
            "wv": wv.astype(NPBF16),
            "wo": wo,
        }
        if causal:
            m["bm"] = bm
        else:
            # transposed multiplicative mask per local head: mt[h, k, q]
            mt = np.ascontiguousarray(
                mask[b, h0 : h0 + HG].transpose(0, 2, 1)
            ).astype(NPBF16)
            m["mt"] = mt
        in_maps.append(m)
    return in_maps


def kernel(**inputs):
    from concourse.bass_utils import run_bass_kernel_spmd

    x = np.asarray(inputs["x"], dtype=np.float32)
    mask = np.asarray(inputs["mask"])
    Wq = np.asarray(inputs["Wq"], dtype=np.float32)
    bq = np.asarray(inputs["bq"], dtype=np.float32)
    Wk = np.asarray(inputs["Wk"], dtype=np.float32)
    Wv = np.asarray(inputs["Wv"], dtype=np.float32)
    bv = np.asarray(inputs["bv"], dtype=np.float32)
    Wo = np.asarray(inputs["Wo"], dtype=np.float32)
    bo = np.asarray(inputs["bo"], dtype=np.float32)
    # bk is softmax-invariant (adds a per-query constant to all logits in a
    # row), so it is deliberately not used.

    causal = bool(
        (mask == np.tril(np.ones((S, S), dtype=bool))[None, None]).all()
    )

    nc = _get_nc(causal)
    in_maps = _prep_core_inputs(x, mask, Wq, bq, Wk, Wv, bv, Wo, causal)
    res = run_bass_kernel_spmd(nc, in_maps, core_ids=list(range(8)))
    out = np.empty((B, S, D), np.float32)
    for b in range(B):
        partial = res.results[2 * b]["out"] + res.results[2 * b + 1]["out"]
        out[b] = partial.T + bo[None, :]
    return out
